# revision 15
# baseline (speedup 1.0000x reference)
"""GAT (2-layer, 4-head then 1-head) on 8 Trainium2 NeuronCores.

Strategy (v3 — dense one-hot chunks, group-level batching)
----------------------------------------------------------
- Nodes degree-sorted and dealt round-robin to 8 cores; each core's 5120
  nodes form 40 dst tiles of 128.
- Edges (self-loops excluded) are packed DENSELY per (tile, table-half)
  into 128-edge chunks (pad ~9%). Aggregation multiplies each chunk by a
  one-hot lhsT whose column d selects the partitions holding edges of
  dst d, accumulating Sum_e w_e*[h_e | 1] per dst in PSUM.
- Per-edge al_dst: alD[p] = onehotT_c (contract dst) al8_own — computed
  for ALL chunks of a group into one PSUM bank, one matmul per chunk.
  onehotT is streamed from DRAM; the aggregation one-hot is generated on
  device (DVE is_equal(codes, iota)).
- All per-edge DVE work (al add, exp, max, message multiply) happens at
  GROUP granularity (~36 chunks per instruction), not per tile; only the
  aggregation matmuls and the epilogue are per tile.
- Self-loops never gathered: extra rhs slots + identity-matmul chunks
  fed from SBUF-resident h_own / z_own.
- Softmax without max-subtraction (exp(lrelu(x)) = max(exp x, exp .2x));
  denominators ride the same one-hot matmul (w columns). Pad slots have
  all-zero one-hot columns.
- Node tables: L1 rows [h(128)|al_src(4)|pad] 512B; L2 rows
  [z+b2(32)|as2(1)|pad] 256B. Table rows are numbered CHUNK-MAJOR
  (pos<2560 first for all cores, then the rest) so each half-table
  AllGather has a contiguous output and can overlap compute. int16
  gather indices via the A/B table split at row 30721.
"""

import numpy as np

import concourse.bacc as bacc
import concourse.mybir as mybir
import concourse.tile as tile
from concourse.bass_utils import run_bass_kernel_spmd

F32 = mybir.dt.float32
F16 = mybir.dt.float16
I16 = mybir.dt.int16

IN_CH = 128
HID = 32
HEADS = 4
OUT_CH = 112
NEG_SLOPE = 0.2

T1_COLS = 256
T2_COLS = 128

N_CORES = 8
GCAP = 28
PAD_CODE = 200
HROWS = 2560  # rows per core per allgather chunk (2 chunks)


def _prep(x, edge_index, W1, a_src1, a_dst1, b1, W2, a_src2, a_dst2, b2, W_out, b_out):
    N = x.shape[0]
    per_core = -(-N // (N_CORES * 128)) * 128
    n_pad = per_core * N_CORES
    NT = per_core // 128
    nrows = n_pad + 2
    b_base = 1 + N_CORES * HROWS
    assert b_base - 1 <= 32767 and nrows - b_base <= 32767
    assert per_core == 2 * HROWS

    src = np.asarray(edge_index[0], np.int64)
    dst = np.asarray(edge_index[1], np.int64)
    E = src.shape[0]

    deg = np.bincount(dst, minlength=n_pad)
    order = np.argsort(deg, kind="stable")
    rank = np.empty(n_pad, np.int64)
    rank[order] = np.arange(n_pad)
    coreid = rank % N_CORES
    pos = rank // N_CORES
    grow = coreid * per_core + pos
    # chunk-major table rows: all cores' pos<HROWS first, then the rest
    trow = 1 + (pos // HROWS) * (N_CORES * HROWS) + coreid * HROWS + pos % HROWS
    perm_rows = np.empty(n_pad, np.int64)
    perm_rows[grow] = np.arange(n_pad)

    sr = trow[src]
    dr = grow[dst]
    gB = sr >= b_base
    core = dr // per_core
    tl = (dr % per_core) // 128
    lane = dr % 128

    EaT = np.zeros((N_CORES, NT), np.int64)
    EbT = np.zeros((N_CORES, NT), np.int64)
    np.add.at(EaT, (core[~gB], tl[~gB]), 1)
    np.add.at(EbT, (core[gB], tl[gB]), 1)
    chA = (-(-EaT // 128)).max(axis=0)
    chB = (-(-EbT // 128)).max(axis=0)

    groups = []
    t = 0
    while t < NT:
        e = t
        tot = 0
        while e < NT and (e == t or tot + chA[e] + chB[e] <= GCAP):
            tot += chA[e] + chB[e]
            e += 1
        groups.append((t, e))
        t = e

    aoff = np.zeros(NT, np.int64)
    boff = np.zeros(NT, np.int64)
    gc0 = []
    gSA = []
    gSB = []
    C = 0
    g_of_tile = np.zeros(NT, np.int64)
    for gi, (t0, t1) in enumerate(groups):
        sa = int(chA[t0:t1].sum())
        sb = int(chB[t0:t1].sum())
        gc0.append(C)
        gSA.append(sa)
        gSB.append(sb)
        off = 0
        for t in range(t0, t1):
            g_of_tile[t] = gi
            aoff[t] = off
            off += chA[t]
        off = 0
        for t in range(t0, t1):
            boff[t] = off
            off += chB[t]
        C += sa + sb
    totidx = C * 128
    assert totidx % 16 == 0

    SENT_A = 0
    SENT_B = nrows - 1 - b_base
    idx_streams = np.zeros((N_CORES, C, 128), np.int16)
    for gi in range(len(groups)):
        idx_streams[:, gc0[gi]:gc0[gi] + gSA[gi], :] = SENT_A
        idx_streams[:, gc0[gi] + gSA[gi]:gc0[gi] + gSA[gi] + gSB[gi], :] = SENT_B
    codes_streams = np.full((N_CORES, C, 128), PAD_CODE, np.int16)

    cbaseA = np.array([gc0[g_of_tile[t]] + aoff[t] for t in range(NT)])
    cbaseB = np.array([gc0[g_of_tile[t]] + gSA[g_of_tile[t]] + boff[t] for t in range(NT)])

    key = (core * NT + tl) * 2 + gB.astype(np.int64)
    eorder = np.argsort(key, kind="stable")
    ks = key[eorder]
    newrun = np.ones(E, bool)
    newrun[1:] = ks[1:] != ks[:-1]
    run_start = np.flatnonzero(newrun)
    run_id = np.cumsum(newrun) - 1
    j = np.arange(E) - run_start[run_id]
    cs = core[eorder]
    tls = tl[eorder]
    gs = gB[eorder]
    cidx = np.where(gs, cbaseB[tls], cbaseA[tls]) + j // 128
    idx_streams[cs, cidx, j % 128] = np.where(gs, sr[eorder] - b_base, sr[eorder]).astype(np.int16)
    codes_streams[cs, cidx, j % 128] = lane[eorder]

    idx_wrapped = np.empty((N_CORES, 128, totidx // 16), np.int16)
    for c in range(N_CORES):
        w16 = idx_streams[c].reshape(-1, 16).T
        idx_wrapped[c] = np.tile(w16, (8, 1))

    f16 = np.float16
    codes_pc = np.transpose(codes_streams, (0, 2, 1)).astype(f16)
    d_ar = np.arange(128, dtype=np.int16)[:, None, None]
    ohT = np.empty((N_CORES, 128, C, 128), f16)
    for c in range(N_CORES):
        ohT[c] = (codes_streams[c][None, :, :] == d_ar).astype(f16)

    xp = np.zeros((n_pad, IN_CH), np.float32)
    xp[:N] = np.asarray(x, np.float32)
    x_slices = np.empty((N_CORES, IN_CH, per_core), np.float32)
    for c in range(N_CORES):
        x_slices[c] = xp[perm_rows[c * per_core:(c + 1) * per_core]].T

    W1 = np.asarray(W1, np.float32)
    Bsrc = np.zeros((HEADS * HID, HEADS), np.float32)
    Bdst = np.zeros((HEADS * HID, HEADS), np.float32)
    for h in range(HEADS):
        Bsrc[h * HID:(h + 1) * HID, h] = np.asarray(a_src1[h], np.float32)
        Bdst[h * HID:(h + 1) * HID, h] = np.asarray(a_dst1[h], np.float32)
    W1big = np.concatenate([W1, W1 @ Bsrc, W1 @ Bdst], axis=1)
    W2 = np.asarray(W2, np.float32)
    W2big = np.concatenate(
        [W2, W2 @ np.asarray(a_src2, np.float32).T, W2 @ np.asarray(a_dst2, np.float32).T],
        axis=1,
    )
    b1_rep = np.tile(np.asarray(b1, np.float32)[None, :], (128, 1))
    b2_rep = np.zeros((128, HID + 2), np.float32)
    b2_rep[:, :HID] = np.asarray(b2, np.float32)[None, :]
    bout_rep = np.tile(np.asarray(b_out, np.float32)[None, :], (128, 1))
    ident = np.eye(128, dtype=f16)
    iota = np.tile(np.arange(128, dtype=f16)[None, :], (128, 1))

    meta = dict(
        N=N, n_pad=n_pad, per_core=per_core, NT=NT, nrows=nrows, b_base=b_base,
        chA=chA.tolist(), chB=chB.tolist(), groups=groups, gc0=gc0, gSA=gSA,
        gSB=gSB, aoff=aoff.tolist(), boff=boff.tolist(), C=C, totidx=totidx,
        perm_rows=perm_rows,
    )
    shared = dict(
        W1big=W1big, W2big=W2big.astype(f16), Wout=np.asarray(W_out, np.float32).astype(f16),
        b1_rep=b1_rep, b2_rep=b2_rep, bout_rep=bout_rep, ident=ident, iota=iota,
    )
    in_maps = []
    for c in range(N_CORES):
        m = dict(shared)
        m["x_slice"] = np.ascontiguousarray(x_slices[c])
        m["idx_flat"] = np.ascontiguousarray(idx_wrapped[c])
        m["codes"] = np.ascontiguousarray(codes_pc[c])
        m["ohT"] = np.ascontiguousarray(ohT[c])
        in_maps.append(m)
    return meta, in_maps


def _build(meta):
    per_core, NT, nrows, b_base = meta["per_core"], meta["NT"], meta["nrows"], meta["b_base"]
    chA, chB = meta["chA"], meta["chB"]
    groups, gc0, gSA, gSB = meta["groups"], meta["gc0"], meta["gSA"], meta["gSB"]
    aoff, boff, C = meta["aoff"], meta["boff"], meta["C"]
    totidx = meta["totidx"]

    nc = bacc.Bacc("TRN2", num_devices=N_CORES, num_swdge_queues=4,
                   dynamic_dma_scratch_size=32768)

    x_slice = nc.dram_tensor("x_slice", [IN_CH, per_core], F32, kind="ExternalInput")
    idx_flat = nc.dram_tensor("idx_flat", [128, totidx // 16], I16, kind="ExternalInput")
    codes_d = nc.dram_tensor("codes", [128, C], F16, kind="ExternalInput")
    ohT_d = nc.dram_tensor("ohT", [128, C, 128], F16, kind="ExternalInput")
    W1big_d = nc.dram_tensor("W1big", [128, 136], F32, kind="ExternalInput")
    W2big_d = nc.dram_tensor("W2big", [128, HID + 2], F16, kind="ExternalInput")
    Wout_d = nc.dram_tensor("Wout", [HID, OUT_CH], F16, kind="ExternalInput")
    b1_d = nc.dram_tensor("b1_rep", [128, 128], F32, kind="ExternalInput")
    b2_d = nc.dram_tensor("b2_rep", [128, HID + 2], F32, kind="ExternalInput")
    bout_d = nc.dram_tensor("bout_rep", [128, OUT_CH], F32, kind="ExternalInput")
    ident_d = nc.dram_tensor("ident", [128, 128], F16, kind="ExternalInput")
    iota_d = nc.dram_tensor("iota", [128, 128], F16, kind="ExternalInput")

    T1_own = nc.dram_tensor("T1_own", [per_core, T1_COLS], F16, kind="Internal")
    T1_sh = nc.dram_tensor("T1_sh", [nrows, T1_COLS], F16, kind="Internal", addr_space="Shared")
    T2_own = nc.dram_tensor("T2_own", [per_core, T2_COLS], F16, kind="Internal")
    T2_sh = nc.dram_tensor("T2_sh", [nrows, T2_COLS], F16, kind="Internal", addr_space="Shared")
    out_d = nc.dram_tensor("out", [per_core, OUT_CH], F32, kind="ExternalOutput")

    rgroups = [list(range(N_CORES))]
    qctr = [0]

    def qn():
        q = qctr[0] % 4
        qctr[0] += 1
        return q

    def allgather(own, sh, half):
        r0 = half * HROWS
        o0 = 1 + half * N_CORES * HROWS
        nc.gpsimd.collective_compute(
            "AllGather", mybir.AluOpType.bypass, replica_groups=rgroups,
            ins=[own[r0:r0 + HROWS, :]], outs=[sh[o0:o0 + N_CORES * HROWS, :]],
        )

    # per-group slot -> tile map
    slot_tile = []
    for gi, (t0, t1) in enumerate(groups):
        st = []
        for t in range(t0, t1):
            st += [t] * chA[t]
        for t in range(t0, t1):
            st += [t] * chB[t]
        slot_tile.append(st)

    with tile.TileContext(nc) as tc:
        with (
            tc.tile_pool(name="const", bufs=1) as cp,
            tc.tile_pool(name="persist", bufs=1) as pp,
            tc.tile_pool(name="xa", bufs=2) as xap,
            tc.tile_pool(name="stage", bufs=3) as sp,
            tc.tile_pool(name="idxp", bufs=3) as ixp,
            tc.tile_pool(name="gath", bufs=3) as gp,
            tc.tile_pool(name="ot", bufs=3) as otp,
            tc.tile_pool(name="oh", bufs=3) as ohp,
            tc.tile_pool(name="rhs", bufs=3) as rp,
            tc.tile_pool(name="small", bufs=3) as smp,
            tc.tile_pool(name="epi", bufs=2) as ep,
            tc.tile_pool(name="psa", bufs=3, space="PSUM") as ppa,
            tc.tile_pool(name="psal", bufs=2, space="PSUM") as pal,
            tc.tile_pool(name="psm", bufs=3, space="PSUM") as ppm,
        ):
            # ---- consts
            W1big = cp.tile([128, 136], F32)
            nc.sync.dma_start(out=W1big[:], in_=W1big_d[:])
            W2big = cp.tile([128, HID + 2], F16)
            nc.sync.dma_start(out=W2big[:], in_=W2big_d[:])
            Wout = cp.tile([HID, OUT_CH], F16)
            nc.sync.dma_start(out=Wout[:], in_=Wout_d[:])
            b1r = cp.tile([128, 128], F32)
            nc.sync.dma_start(out=b1r[:], in_=b1_d[:])
            b2r = cp.tile([128, HID + 2], F32)
            nc.sync.dma_start(out=b2r[:], in_=b2_d[:])
            boutr = cp.tile([128, OUT_CH], F32)
            nc.sync.dma_start(out=boutr[:], in_=bout_d[:])
            ident = cp.tile([128, 128], F16)
            nc.sync.dma_start(out=ident[:], in_=ident_d[:])
            iota = cp.tile([128, 128], F16)
            nc.sync.dma_start(out=iota[:], in_=iota_d[:])
            codes = cp.tile([128, C], F16)
            nc.sync.dma_start(out=codes[:], in_=codes_d[:])

            h_own = pp.tile([128, NT * 128], F16)
            alT1 = pp.tile([128, NT * 8], F16)
            z_own = pp.tile([128, NT * HID], F16)
            alT2 = pp.tile([128, NT * 2], F16)
            wself1 = pp.tile([128, NT, HEADS], F16)
            wself2 = pp.tile([128, NT, 1], F16)

            zs1 = cp.tile([1, T1_COLS], F16)
            nc.vector.memset(zs1[:], 0.0)
            nc.sync.dma_start(out=T1_sh[0:1, :], in_=zs1[:])
            nc.sync.dma_start(out=T1_sh[nrows - 1:nrows, :], in_=zs1[:])
            nc.sync.dma_start(out=T2_sh[0:1, :], in_=zs1[:, 0:T2_COLS])
            nc.sync.dma_start(out=T2_sh[nrows - 1:nrows, :], in_=zs1[:, 0:T2_COLS])

            # ---- phase A
            for t in range(NT):
                xa = xap.tile([128, 128], F32)
                nc.sync.dma_start(out=xa[:], in_=x_slice[:, t * 128:(t + 1) * 128])
                ps = ppa.tile([128, 136], F32, tag="agg")
                nc.tensor.matmul(out=ps[:], lhsT=xa[:], rhs=W1big[:], start=True, stop=True)
                hb = sp.tile([128, T1_COLS], F16, tag="hb")
                nc.vector.tensor_copy(out=hb[:, 0:132], in_=ps[:, 0:132])
                nc.scalar.dma_start(out=T1_own[t * 128:(t + 1) * 128, :], in_=hb[:])
                nc.vector.tensor_copy(out=h_own[:, t * 128:(t + 1) * 128], in_=ps[:, 0:128])
                nc.vector.tensor_copy(out=alT1[:, t * 8:t * 8 + 8], in_=ps[:, 128:136])
                if t == NT // 2 - 1:
                    allgather(T1_own, T1_sh, 0)
            allgather(T1_own, T1_sh, 1)

            alT1v = alT1[:].rearrange("p (t e) -> p t e", t=NT)
            xls = smp.tile([128, NT, HEADS], F16, tag="xls")
            nc.vector.tensor_tensor(out=xls[:], in0=alT1v[:, :, 0:4], in1=alT1v[:, :, 4:8],
                                    op=mybir.AluOpType.add)
            e1s = smp.tile([128, NT, HEADS], F16, tag="e1s")
            nc.scalar.activation(e1s[:], xls[:], mybir.ActivationFunctionType.Exp)
            e2s = smp.tile([128, NT, HEADS], F16, tag="e2s")
            nc.scalar.activation(e2s[:], xls[:], mybir.ActivationFunctionType.Exp, scale=NEG_SLOPE)
            nc.vector.tensor_tensor(out=wself1[:], in0=e1s[:], in1=e2s[:], op=mybir.AluOpType.max)

            # ---- layer 1
            for gi, (t0, t1) in enumerate(groups):
                SA, SB = gSA[gi], gSB[gi]
                S = SA + SB
                nt = t1 - t0
                c0 = gc0[gi]
                idxg = ixp.tile([128, S * 8], I16, tag="idx")
                nc.scalar.dma_start(out=idxg[:], in_=idx_flat[:, c0 * 8:(c0 + S) * 8])
                G = gp.tile([128, S, T1_COLS], F16, tag="G1")
                if SA:
                    nc.gpsimd.dma_gather(
                        G[:, 0:SA, :], T1_sh[0:b_base, :], idxg[:, 0:SA * 8],
                        128 * SA, 128 * SA, T1_COLS, queue_num=qn(), single_packet=False)
                if SB:
                    nc.gpsimd.dma_gather(
                        G[:, SA:S, :], T1_sh[b_base:nrows, :], idxg[:, SA * 8:S * 8],
                        128 * SB, 128 * SB, T1_COLS, queue_num=qn(), single_packet=False)
                OT = otp.tile([128, S, 128], F16, tag="OT")
                nc.scalar.dma_start(out=OT[:], in_=ohT_d[:, c0:c0 + S, :])
                OH = ohp.tile([128, S, 128], F16, tag="OH")
                nc.vector.tensor_tensor(
                    out=OH[:], in0=codes[:, c0:c0 + S, None].to_broadcast([128, S, 128]),
                    in1=iota[:, None, :].to_broadcast([128, S, 128]),
                    op=mybir.AluOpType.is_equal)

                # group-level alD / weights / messages
                alps = pal.tile([128, 4 * S], F32, tag="al")
                for cs_ in range(S):
                    t = slot_tile[gi][cs_]
                    nc.tensor.matmul(out=alps[:, 4 * cs_:4 * cs_ + 4], lhsT=OT[:, cs_, :],
                                     rhs=alT1[:, t * 8 + 4:t * 8 + 8], start=True, stop=True)
                alDs = smp.tile([128, S, HEADS], F16, tag="alDs")
                nc.vector.tensor_copy(out=alDs[:], in_=alps[:].rearrange("p (k e) -> p k e", e=4))
                xl = smp.tile([128, S, HEADS], F16, tag="xl")
                nc.vector.tensor_tensor(out=xl[:], in0=G[:, :, 128:132], in1=alDs[:],
                                        op=mybir.AluOpType.add)
                e1 = smp.tile([128, S, HEADS], F16, tag="e1")
                nc.scalar.activation(e1[:], xl[:], mybir.ActivationFunctionType.Exp)
                e2 = smp.tile([128, S, HEADS], F16, tag="e2")
                nc.scalar.activation(e2[:], xl[:], mybir.ActivationFunctionType.Exp, scale=NEG_SLOPE)
                rhs = rp.tile([128, S + nt, 132], F16, tag="rhs1")
                nc.vector.tensor_tensor(out=rhs[:, 0:S, 128:132], in0=e1[:], in1=e2[:],
                                        op=mybir.AluOpType.max)
                nc.vector.tensor_tensor(
                    out=rhs[:, 0:S, 0:128].rearrange("p k (h j) -> p k h j", h=4),
                    in0=G[:, :, 0:128].rearrange("p k (h j) -> p k h j", h=4),
                    in1=rhs[:, 0:S, 128:132][:, :, :, None].to_broadcast([128, S, 4, 32]),
                    op=mybir.AluOpType.mult)
                nc.vector.tensor_copy(
                    out=rhs[:, S:S + nt, 128:132], in_=wself1[:, t0:t1, :])
                nc.vector.tensor_tensor(
                    out=rhs[:, S:S + nt, 0:128].rearrange("p k (h j) -> p k h j", h=4),
                    in0=h_own[:, t0 * 128:t1 * 128].rearrange("p (t h j) -> p t h j", t=nt, h=4),
                    in1=wself1[:, t0:t1, :, None].to_broadcast([128, nt, 4, 32]),
                    op=mybir.AluOpType.mult)

                psall = ep.tile([128, nt, 132], F32, tag="psall")
                for ti, t in enumerate(range(t0, t1)):
                    ca, cb = chA[t], chB[t]
                    slots = list(range(aoff[t], aoff[t] + ca)) + \
                            list(range(SA + boff[t], SA + boff[t] + cb))
                    ps = ppa.tile([128, 132], F32, tag="agg")
                    for ci, cs_ in enumerate(slots):
                        nc.tensor.matmul(out=ps[:], lhsT=OH[:, cs_, :], rhs=rhs[:, cs_, :],
                                         start=(ci == 0), stop=False)
                    nc.tensor.matmul(out=ps[:], lhsT=ident[:], rhs=rhs[:, S + ti, :],
                                     start=False, stop=True)
                    nc.vector.tensor_copy(out=psall[:, ti, :], in_=ps[:])
                psg_v = psall[:]

                # group-level epilogue
                rec = smp.tile([128, nt, HEADS], F32, tag="rec")
                nc.vector.reciprocal(out=rec[:], in_=psg_v[:, :, 128:132])
                y1 = ep.tile([128, nt, 128], F16, tag="y")
                nc.vector.tensor_tensor(
                    out=y1[:].rearrange("p t (h j) -> p t h j", h=4),
                    in0=psg_v[:, :, 0:128].rearrange("p t (h j) -> p t h j", h=4),
                    in1=rec[:, :, :, None].to_broadcast([128, nt, 4, 32]),
                    op=mybir.AluOpType.mult)
                nc.vector.tensor_tensor(
                    out=y1[:], in0=y1[:],
                    in1=b1r[:, None, :].to_broadcast([128, nt, 128]),
                    op=mybir.AluOpType.add)
                m1 = ep.tile([128, nt, 128], F16, tag="m1")
                nc.vector.tensor_scalar(out=m1[:], in0=y1[:], scalar1=0.0, scalar2=None,
                                        op0=mybir.AluOpType.min)
                nc.scalar.activation(m1[:], m1[:], mybir.ActivationFunctionType.Exp)
                nc.vector.tensor_scalar(out=y1[:], in0=y1[:], scalar1=0.0, scalar2=-1.0,
                                        op0=mybir.AluOpType.max, op1=mybir.AluOpType.add)
                h2 = m1
                nc.vector.tensor_tensor(out=h2[:], in0=m1[:], in1=y1[:], op=mybir.AluOpType.add)

                t2g = sp.tile([128, nt, T2_COLS], F16, tag="t2b")
                for ti, t in enumerate(range(t0, t1)):
                    pt = ppm.tile([128, 128], F16, tag="misc")
                    nc.tensor.transpose(out=pt[:], in_=h2[:, ti, :], identity=ident[:])
                    h2T = ep.tile([128, 128], F16, tag="h2T")
                    nc.vector.tensor_copy(out=h2T[:], in_=pt[:])
                    psz = ppm.tile([128, HID + 2], F32, tag="misc")
                    nc.tensor.matmul(out=psz[:], lhsT=h2T[:], rhs=W2big[:], start=True, stop=True)
                    nc.vector.tensor_tensor(out=t2g[:, ti, 0:HID + 2], in0=psz[:], in1=b2r[:],
                                            op=mybir.AluOpType.add)
                nc.scalar.dma_start(
                    out=T2_own[t0 * 128:t1 * 128, :].rearrange("(t p) c -> p t c", p=128),
                    in_=t2g[:])
                nc.vector.tensor_copy(
                    out=z_own[:, t0 * HID:t1 * HID].rearrange("p (t c) -> p t c", t=nt),
                    in_=t2g[:, :, 0:HID])
                nc.vector.tensor_copy(
                    out=alT2[:, t0 * 2:t1 * 2].rearrange("p (t c) -> p t c", t=nt),
                    in_=t2g[:, :, HID:HID + 2])
                if t0 < NT // 2 <= t1:
                    allgather(T2_own, T2_sh, 0)
            allgather(T2_own, T2_sh, 1)

            alT2v = alT2[:].rearrange("p (t e) -> p t e", t=NT)
            xls2 = smp.tile([128, NT, 1], F16, tag="xls2")
            nc.vector.tensor_tensor(out=xls2[:], in0=alT2v[:, :, 0:1], in1=alT2v[:, :, 1:2],
                                    op=mybir.AluOpType.add)
            e1s2 = smp.tile([128, NT, 1], F16, tag="e1s2")
            nc.scalar.activation(e1s2[:], xls2[:], mybir.ActivationFunctionType.Exp)
            e2s2 = smp.tile([128, NT, 1], F16, tag="e2s2")
            nc.scalar.activation(e2s2[:], xls2[:], mybir.ActivationFunctionType.Exp, scale=NEG_SLOPE)
            nc.vector.tensor_tensor(out=wself2[:], in0=e1s2[:], in1=e2s2[:], op=mybir.AluOpType.max)

            # ---- layer 2
            for gi, (t0, t1) in enumerate(groups):
                SA, SB = gSA[gi], gSB[gi]
                S = SA + SB
                nt = t1 - t0
                c0 = gc0[gi]
                idxg = ixp.tile([128, S * 8], I16, tag="idx")
                nc.scalar.dma_start(out=idxg[:], in_=idx_flat[:, c0 * 8:(c0 + S) * 8])
                G2 = gp.tile([128, S, T2_COLS], F16, tag="G2")
                if SA:
                    nc.gpsimd.dma_gather(
                        G2[:, 0:SA, :], T2_sh[0:b_base, :], idxg[:, 0:SA * 8],
                        128 * SA, 128 * SA, T2_COLS, queue_num=qn(), single_packet=False)
                if SB:
                    nc.gpsimd.dma_gather(
                        G2[:, SA:S, :], T2_sh[b_base:nrows, :], idxg[:, SA * 8:S * 8],
                        128 * SB, 128 * SB, T2_COLS, queue_num=qn(), single_packet=False)
                OT = otp.tile([128, S, 128], F16, tag="OT")
                nc.scalar.dma_start(out=OT[:], in_=ohT_d[:, c0:c0 + S, :])
                OH = ohp.tile([128, S, 128], F16, tag="OH")
                nc.vector.tensor_tensor(
                    out=OH[:], in0=codes[:, c0:c0 + S, None].to_broadcast([128, S, 128]),
                    in1=iota[:, None, :].to_broadcast([128, S, 128]),
                    op=mybir.AluOpType.is_equal)

                alps2 = pal.tile([128, S], F32, tag="al")
                for cs_ in range(S):
                    t = slot_tile[gi][cs_]
                    nc.tensor.matmul(out=alps2[:, cs_:cs_ + 1], lhsT=OT[:, cs_, :],
                                     rhs=alT2[:, t * 2 + 1:t * 2 + 2], start=True, stop=True)
                alDs2 = smp.tile([128, S, 1], F16, tag="alDs2")
                nc.vector.tensor_copy(out=alDs2[:], in_=alps2[:, :, None])
                xl2 = smp.tile([128, S, 1], F16, tag="xl2")
                nc.vector.tensor_tensor(out=xl2[:], in0=G2[:, :, 32:33], in1=alDs2[:],
                                        op=mybir.AluOpType.add)
                e1b = smp.tile([128, S, 1], F16, tag="e1b")
                nc.scalar.activation(e1b[:], xl2[:], mybir.ActivationFunctionType.Exp)
                e2b = smp.tile([128, S, 1], F16, tag="e2b")
                nc.scalar.activation(e2b[:], xl2[:], mybir.ActivationFunctionType.Exp, scale=NEG_SLOPE)
                rhs2 = rp.tile([128, S + nt, HID + 1], F16, tag="rhs2")
                nc.vector.tensor_tensor(out=rhs2[:, 0:S, HID:HID + 1], in0=e1b[:], in1=e2b[:],
                                        op=mybir.AluOpType.max)
                nc.vector.tensor_tensor(
                    out=rhs2[:, 0:S, 0:HID], in0=G2[:, :, 0:HID],
                    in1=rhs2[:, 0:S, HID:HID + 1].to_broadcast([128, S, HID]),
                    op=mybir.AluOpType.mult)
                nc.vector.tensor_copy(
                    out=rhs2[:, S:S + nt, HID:HID + 1], in_=wself2[:, t0:t1, :])
                nc.vector.tensor_tensor(
                    out=rhs2[:, S:S + nt, 0:HID],
                    in0=z_own[:, t0 * HID:t1 * HID].rearrange("p (t c) -> p t c", t=nt),
                    in1=wself2[:, t0:t1, :].to_broadcast([128, nt, HID]),
                    op=mybir.AluOpType.mult)

                psall2 = ep.tile([128, nt, 33], F32, tag="psall2")
                for ti, t in enumerate(range(t0, t1)):
                    ca, cb = chA[t], chB[t]
                    slots = list(range(aoff[t], aoff[t] + ca)) + \
                            list(range(SA + boff[t], SA + boff[t] + cb))
                    ps2 = ppa.tile([128, 33], F32, tag="agg")
                    for ci, cs_ in enumerate(slots):
                        nc.tensor.matmul(out=ps2[:], lhsT=OH[:, cs_, :], rhs=rhs2[:, cs_, :],
                                         start=(ci == 0), stop=False)
                    nc.tensor.matmul(out=ps2[:], lhsT=ident[:], rhs=rhs2[:, S + ti, :],
                                     start=False, stop=True)
                    nc.vector.tensor_copy(out=psall2[:, ti, :], in_=ps2[:])
                psg2_v = psall2[:]

                rec2 = smp.tile([128, nt, 1], F32, tag="rec2")
                nc.vector.reciprocal(out=rec2[:], in_=psg2_v[:, :, HID:HID + 1])
                y2 = ep.tile([128, nt, HID], F16, tag="y2")
                nc.vector.tensor_tensor(out=y2[:], in0=psg2_v[:, :, 0:HID],
                                        in1=rec2[:].to_broadcast([128, nt, HID]),
                                        op=mybir.AluOpType.mult)
                m2 = ep.tile([128, nt, HID], F16, tag="m2")
                nc.vector.tensor_scalar(out=m2[:], in0=y2[:], scalar1=0.0, scalar2=None,
                                        op0=mybir.AluOpType.min)
                nc.scalar.activation(m2[:], m2[:], mybir.ActivationFunctionType.Exp)
                nc.vector.tensor_scalar(out=y2[:], in0=y2[:], scalar1=0.0, scalar2=-1.0,
                                        op0=mybir.AluOpType.max, op1=mybir.AluOpType.add)
                h3 = m2
                nc.vector.tensor_tensor(out=h3[:], in0=m2[:], in1=y2[:], op=mybir.AluOpType.add)

                outg = ep.tile([128, nt, OUT_CH], F32, tag="outf")
                for ti, t in enumerate(range(t0, t1)):
                    pt2 = ppm.tile([128, 128], F16, tag="misc")
                    nc.tensor.transpose(out=pt2[:HID, :], in_=h3[:, ti, :], identity=ident[:])
                    h3T = ep.tile([HID, 128], F16, tag="h3T")
                    nc.vector.tensor_copy(out=h3T[:], in_=pt2[:HID, :])
                    psf = ppm.tile([128, OUT_CH], F32, tag="misc")
                    nc.tensor.matmul(out=psf[:], lhsT=h3T[:], rhs=Wout[:], start=True, stop=True)
                    nc.vector.tensor_tensor(out=outg[:, ti, :], in0=psf[:], in1=boutr[:],
                                            op=mybir.AluOpType.add)
                nc.scalar.dma_start(
                    out=out_d[t0 * 128:t1 * 128, :].rearrange("(t p) c -> p t c", p=128),
                    in_=outg[:])

    nc.compile()
    return nc


def _run(inputs, trace=False):
    meta, in_maps = _prep(**inputs)
    nc = _build(meta)
    res = run_bass_kernel_spmd(nc, in_maps, core_ids=list(range(N_CORES)), trace=trace)
    outg = np.concatenate([res.results[c]["out"] for c in range(N_CORES)], axis=0)
    out_nodes = np.empty((meta["n_pad"], OUT_CH), np.float32)
    out_nodes[meta["perm_rows"]] = outg
    return out_nodes[:meta["N"]], res


def kernel(**inputs):
    out, _ = _run(inputs, trace=False)
    return out


# revision 16
# speedup vs baseline: 1.1592x; 1.1592x over previous
"""GAT (2-layer, 4-head then 1-head) on 8 Trainium2 NeuronCores.

Strategy (v3 — dense one-hot chunks, group-level batching)
----------------------------------------------------------
- Nodes degree-sorted and dealt round-robin to 8 cores; each core's 5120
  nodes form 40 dst tiles of 128.
- Edges (self-loops excluded) are packed DENSELY per (tile, table-half)
  into 128-edge chunks (pad ~9%). Aggregation multiplies each chunk by a
  one-hot lhsT whose column d selects the partitions holding edges of
  dst d, accumulating Sum_e w_e*[h_e | 1] per dst in PSUM.
- Per-edge al_dst: alD[p] = onehotT_c (contract dst) al8_own — computed
  for ALL chunks of a group into one PSUM bank, one matmul per chunk.
  onehotT is streamed from DRAM; the aggregation one-hot is generated on
  device (DVE is_equal(codes, iota)).
- All per-edge DVE work (al add, exp, max, message multiply) happens at
  GROUP granularity (~36 chunks per instruction), not per tile; only the
  aggregation matmuls and the epilogue are per tile.
- Self-loops never gathered: extra rhs slots + identity-matmul chunks
  fed from SBUF-resident h_own / z_own.
- Softmax without max-subtraction (exp(lrelu(x)) = max(exp x, exp .2x));
  denominators ride the same one-hot matmul (w columns). Pad slots have
  all-zero one-hot columns.
- Node tables: L1 rows [h(128)|al_src(4)|pad] 512B; L2 rows
  [z+b2(32)|as2(1)|pad] 256B. Table rows are numbered CHUNK-MAJOR
  (pos<2560 first for all cores, then the rest) so each half-table
  AllGather has a contiguous output and can overlap compute. int16
  gather indices via the A/B table split at row 30721.
"""

import numpy as np

import concourse.bacc as bacc
import concourse.mybir as mybir
import concourse.tile as tile
from concourse.bass_utils import run_bass_kernel_spmd

F32 = mybir.dt.float32
F16 = mybir.dt.float16
I16 = mybir.dt.int16

IN_CH = 128
HID = 32
HEADS = 4
OUT_CH = 112
NEG_SLOPE = 0.2

T1_COLS = 256
T2_COLS = 128

N_CORES = 8
GCAP = 36
PAD_CODE = 200
HROWS = 2560  # rows per core per allgather chunk (2 chunks)


def _prep(x, edge_index, W1, a_src1, a_dst1, b1, W2, a_src2, a_dst2, b2, W_out, b_out):
    N = x.shape[0]
    per_core = -(-N // (N_CORES * 128)) * 128
    n_pad = per_core * N_CORES
    NT = per_core // 128
    nrows = n_pad + 2
    b_base = 1 + N_CORES * HROWS
    assert b_base - 1 <= 32767 and nrows - b_base <= 32767
    assert per_core == 2 * HROWS

    src = np.asarray(edge_index[0], np.int64)
    dst = np.asarray(edge_index[1], np.int64)
    E = src.shape[0]

    deg = np.bincount(dst, minlength=n_pad)
    order = np.argsort(deg, kind="stable")
    rank = np.empty(n_pad, np.int64)
    rank[order] = np.arange(n_pad)
    coreid = rank % N_CORES
    pos = rank // N_CORES
    grow = coreid * per_core + pos
    # chunk-major table rows: all cores' pos<HROWS first, then the rest
    trow = 1 + (pos // HROWS) * (N_CORES * HROWS) + coreid * HROWS + pos % HROWS
    perm_rows = np.empty(n_pad, np.int64)
    perm_rows[grow] = np.arange(n_pad)

    sr = trow[src]
    dr = grow[dst]
    gB = sr >= b_base
    core = dr // per_core
    tl = (dr % per_core) // 128
    lane = dr % 128

    EaT = np.zeros((N_CORES, NT), np.int64)
    EbT = np.zeros((N_CORES, NT), np.int64)
    np.add.at(EaT, (core[~gB], tl[~gB]), 1)
    np.add.at(EbT, (core[gB], tl[gB]), 1)
    chA = (-(-EaT // 128)).max(axis=0)
    chB = (-(-EbT // 128)).max(axis=0)

    groups = []
    t = 0
    while t < NT:
        e = t
        tot = 0
        while e < NT and (e == t or tot + chA[e] + chB[e] <= GCAP):
            tot += chA[e] + chB[e]
            e += 1
        groups.append((t, e))
        t = e

    aoff = np.zeros(NT, np.int64)
    boff = np.zeros(NT, np.int64)
    gc0 = []
    gSA = []
    gSB = []
    C = 0
    g_of_tile = np.zeros(NT, np.int64)
    for gi, (t0, t1) in enumerate(groups):
        sa = int(chA[t0:t1].sum())
        sb = int(chB[t0:t1].sum())
        gc0.append(C)
        gSA.append(sa)
        gSB.append(sb)
        off = 0
        for t in range(t0, t1):
            g_of_tile[t] = gi
            aoff[t] = off
            off += chA[t]
        off = 0
        for t in range(t0, t1):
            boff[t] = off
            off += chB[t]
        C += sa + sb
    totidx = C * 128
    assert totidx % 16 == 0

    SENT_A = 0
    SENT_B = nrows - 1 - b_base
    idx_streams = np.zeros((N_CORES, C, 128), np.int16)
    for gi in range(len(groups)):
        idx_streams[:, gc0[gi]:gc0[gi] + gSA[gi], :] = SENT_A
        idx_streams[:, gc0[gi] + gSA[gi]:gc0[gi] + gSA[gi] + gSB[gi], :] = SENT_B
    codes_streams = np.full((N_CORES, C, 128), PAD_CODE, np.int16)

    cbaseA = np.array([gc0[g_of_tile[t]] + aoff[t] for t in range(NT)])
    cbaseB = np.array([gc0[g_of_tile[t]] + gSA[g_of_tile[t]] + boff[t] for t in range(NT)])

    key = (core * NT + tl) * 2 + gB.astype(np.int64)
    eorder = np.argsort(key, kind="stable")
    ks = key[eorder]
    newrun = np.ones(E, bool)
    newrun[1:] = ks[1:] != ks[:-1]
    run_start = np.flatnonzero(newrun)
    run_id = np.cumsum(newrun) - 1
    j = np.arange(E) - run_start[run_id]
    cs = core[eorder]
    tls = tl[eorder]
    gs = gB[eorder]
    cidx = np.where(gs, cbaseB[tls], cbaseA[tls]) + j // 128
    idx_streams[cs, cidx, j % 128] = np.where(gs, sr[eorder] - b_base, sr[eorder]).astype(np.int16)
    codes_streams[cs, cidx, j % 128] = lane[eorder]

    idx_wrapped = np.empty((N_CORES, 128, totidx // 16), np.int16)
    for c in range(N_CORES):
        w16 = idx_streams[c].reshape(-1, 16).T
        idx_wrapped[c] = np.tile(w16, (8, 1))

    f16 = np.float16
    codes_pc = np.transpose(codes_streams, (0, 2, 1)).astype(f16)
    d_ar = np.arange(128, dtype=np.int16)[:, None, None]
    ohT = np.empty((N_CORES, 128, C, 128), f16)
    for c in range(N_CORES):
        ohT[c] = (codes_streams[c][None, :, :] == d_ar).astype(f16)

    xp = np.zeros((n_pad, IN_CH), np.float32)
    xp[:N] = np.asarray(x, np.float32)
    x_slices = np.empty((N_CORES, IN_CH, per_core), np.float32)
    for c in range(N_CORES):
        x_slices[c] = xp[perm_rows[c * per_core:(c + 1) * per_core]].T

    W1 = np.asarray(W1, np.float32)
    Bsrc = np.zeros((HEADS * HID, HEADS), np.float32)
    Bdst = np.zeros((HEADS * HID, HEADS), np.float32)
    for h in range(HEADS):
        Bsrc[h * HID:(h + 1) * HID, h] = np.asarray(a_src1[h], np.float32)
        Bdst[h * HID:(h + 1) * HID, h] = np.asarray(a_dst1[h], np.float32)
    W1big = np.concatenate([W1, W1 @ Bsrc, W1 @ Bdst], axis=1)
    W2 = np.asarray(W2, np.float32)
    W2big = np.concatenate(
        [W2, W2 @ np.asarray(a_src2, np.float32).T, W2 @ np.asarray(a_dst2, np.float32).T],
        axis=1,
    )
    b1_rep = np.tile(np.asarray(b1, np.float32)[None, :], (128, 1))
    b2_rep = np.zeros((128, HID + 2), np.float32)
    b2_rep[:, :HID] = np.asarray(b2, np.float32)[None, :]
    bout_rep = np.tile(np.asarray(b_out, np.float32)[None, :], (128, 1))
    ident = np.eye(128, dtype=f16)
    iota = np.tile(np.arange(128, dtype=f16)[None, :], (128, 1))

    meta = dict(
        N=N, n_pad=n_pad, per_core=per_core, NT=NT, nrows=nrows, b_base=b_base,
        chA=chA.tolist(), chB=chB.tolist(), groups=groups, gc0=gc0, gSA=gSA,
        gSB=gSB, aoff=aoff.tolist(), boff=boff.tolist(), C=C, totidx=totidx,
        perm_rows=perm_rows,
    )
    shared = dict(
        W1big=W1big, W2big=W2big.astype(f16), Wout=np.asarray(W_out, np.float32).astype(f16),
        b1_rep=b1_rep, b2_rep=b2_rep, bout_rep=bout_rep, ident=ident, iota=iota,
    )
    in_maps = []
    for c in range(N_CORES):
        m = dict(shared)
        m["x_slice"] = np.ascontiguousarray(x_slices[c])
        m["idx_flat"] = np.ascontiguousarray(idx_wrapped[c])
        m["codes"] = np.ascontiguousarray(codes_pc[c])
        m["ohT"] = np.ascontiguousarray(ohT[c])
        in_maps.append(m)
    return meta, in_maps


def _build(meta):
    per_core, NT, nrows, b_base = meta["per_core"], meta["NT"], meta["nrows"], meta["b_base"]
    chA, chB = meta["chA"], meta["chB"]
    groups, gc0, gSA, gSB = meta["groups"], meta["gc0"], meta["gSA"], meta["gSB"]
    aoff, boff, C = meta["aoff"], meta["boff"], meta["C"]
    totidx = meta["totidx"]

    nc = bacc.Bacc("TRN2", num_devices=N_CORES, num_swdge_queues=4,
                   dynamic_dma_scratch_size=32768)

    x_slice = nc.dram_tensor("x_slice", [IN_CH, per_core], F32, kind="ExternalInput")
    idx_flat = nc.dram_tensor("idx_flat", [128, totidx // 16], I16, kind="ExternalInput")
    codes_d = nc.dram_tensor("codes", [128, C], F16, kind="ExternalInput")
    ohT_d = nc.dram_tensor("ohT", [128, C, 128], F16, kind="ExternalInput")
    W1big_d = nc.dram_tensor("W1big", [128, 136], F32, kind="ExternalInput")
    W2big_d = nc.dram_tensor("W2big", [128, HID + 2], F16, kind="ExternalInput")
    Wout_d = nc.dram_tensor("Wout", [HID, OUT_CH], F16, kind="ExternalInput")
    b1_d = nc.dram_tensor("b1_rep", [128, 128], F32, kind="ExternalInput")
    b2_d = nc.dram_tensor("b2_rep", [128, HID + 2], F32, kind="ExternalInput")
    bout_d = nc.dram_tensor("bout_rep", [128, OUT_CH], F32, kind="ExternalInput")
    ident_d = nc.dram_tensor("ident", [128, 128], F16, kind="ExternalInput")
    iota_d = nc.dram_tensor("iota", [128, 128], F16, kind="ExternalInput")

    T1_own = nc.dram_tensor("T1_own", [per_core, T1_COLS], F16, kind="Internal")
    T1_sh = nc.dram_tensor("T1_sh", [nrows, T1_COLS], F16, kind="Internal", addr_space="Shared")
    T2_own = nc.dram_tensor("T2_own", [per_core, T2_COLS], F16, kind="Internal")
    T2_sh = nc.dram_tensor("T2_sh", [nrows, T2_COLS], F16, kind="Internal", addr_space="Shared")
    out_d = nc.dram_tensor("out", [per_core, OUT_CH], F32, kind="ExternalOutput")

    rgroups = [list(range(N_CORES))]
    qctr = [0]

    def qn():
        q = qctr[0] % 4
        qctr[0] += 1
        return q

    def allgather(own, sh, half):
        r0 = half * HROWS
        o0 = 1 + half * N_CORES * HROWS
        nc.gpsimd.collective_compute(
            "AllGather", mybir.AluOpType.bypass, replica_groups=rgroups,
            ins=[own[r0:r0 + HROWS, :]], outs=[sh[o0:o0 + N_CORES * HROWS, :]],
        )

    # per-group slot -> tile map
    slot_tile = []
    for gi, (t0, t1) in enumerate(groups):
        st = []
        for t in range(t0, t1):
            st += [t] * chA[t]
        for t in range(t0, t1):
            st += [t] * chB[t]
        slot_tile.append(st)

    with tile.TileContext(nc) as tc:
        with (
            tc.tile_pool(name="const", bufs=1) as cp,
            tc.tile_pool(name="persist", bufs=1) as pp,
            tc.tile_pool(name="xa", bufs=2) as xap,
            tc.tile_pool(name="stage", bufs=3) as sp,
            tc.tile_pool(name="idxp", bufs=3) as ixp,
            tc.tile_pool(name="gath", bufs=3) as gp,
            tc.tile_pool(name="ot", bufs=2) as otp,
            tc.tile_pool(name="oh", bufs=2) as ohp,
            tc.tile_pool(name="rhs", bufs=2) as rp,
            tc.tile_pool(name="small", bufs=3) as smp,
            tc.tile_pool(name="epi", bufs=2) as ep,
            tc.tile_pool(name="psa", bufs=3, space="PSUM") as ppa,
            tc.tile_pool(name="psal", bufs=2, space="PSUM") as pal,
            tc.tile_pool(name="psm", bufs=3, space="PSUM") as ppm,
        ):
            # ---- consts
            W1big = cp.tile([128, 136], F32)
            nc.sync.dma_start(out=W1big[:], in_=W1big_d[:])
            W2big = cp.tile([128, HID + 2], F16)
            nc.sync.dma_start(out=W2big[:], in_=W2big_d[:])
            Wout = cp.tile([HID, OUT_CH], F16)
            nc.sync.dma_start(out=Wout[:], in_=Wout_d[:])
            b1r = cp.tile([128, 128], F32)
            nc.sync.dma_start(out=b1r[:], in_=b1_d[:])
            b2r = cp.tile([128, HID + 2], F32)
            nc.sync.dma_start(out=b2r[:], in_=b2_d[:])
            boutr = cp.tile([128, OUT_CH], F32)
            nc.sync.dma_start(out=boutr[:], in_=bout_d[:])
            ident = cp.tile([128, 128], F16)
            nc.sync.dma_start(out=ident[:], in_=ident_d[:])
            iota = cp.tile([128, 128], F16)
            nc.sync.dma_start(out=iota[:], in_=iota_d[:])
            codes = cp.tile([128, C], F16)
            nc.sync.dma_start(out=codes[:], in_=codes_d[:])

            h_own = pp.tile([128, NT * 128], F16)
            alT1 = pp.tile([128, NT * 8], F16)
            z_own = pp.tile([128, NT * HID], F16)
            alT2 = pp.tile([128, NT * 2], F16)
            wself1 = pp.tile([128, NT, HEADS], F16)
            wself2 = pp.tile([128, NT, 1], F16)

            zs1 = cp.tile([1, T1_COLS], F16)
            nc.vector.memset(zs1[:], 0.0)
            nc.sync.dma_start(out=T1_sh[0:1, :], in_=zs1[:])
            nc.sync.dma_start(out=T1_sh[nrows - 1:nrows, :], in_=zs1[:])
            nc.sync.dma_start(out=T2_sh[0:1, :], in_=zs1[:, 0:T2_COLS])
            nc.sync.dma_start(out=T2_sh[nrows - 1:nrows, :], in_=zs1[:, 0:T2_COLS])

            # ---- phase A
            for t in range(NT):
                xa = xap.tile([128, 128], F32)
                nc.sync.dma_start(out=xa[:], in_=x_slice[:, t * 128:(t + 1) * 128])
                ps = ppa.tile([128, 136], F32, tag="agg")
                nc.tensor.matmul(out=ps[:], lhsT=xa[:], rhs=W1big[:], start=True, stop=True)
                hb = sp.tile([128, T1_COLS], F16, tag="hb")
                nc.vector.tensor_copy(out=hb[:, 0:132], in_=ps[:, 0:132])
                nc.scalar.dma_start(out=T1_own[t * 128:(t + 1) * 128, :], in_=hb[:])
                nc.vector.tensor_copy(out=h_own[:, t * 128:(t + 1) * 128], in_=ps[:, 0:128])
                nc.vector.tensor_copy(out=alT1[:, t * 8:t * 8 + 8], in_=ps[:, 128:136])
                if t == NT // 2 - 1:
                    allgather(T1_own, T1_sh, 0)
            allgather(T1_own, T1_sh, 1)

            alT1v = alT1[:].rearrange("p (t e) -> p t e", t=NT)
            xls = smp.tile([128, NT, HEADS], F16, tag="xls")
            nc.vector.tensor_tensor(out=xls[:], in0=alT1v[:, :, 0:4], in1=alT1v[:, :, 4:8],
                                    op=mybir.AluOpType.add)
            e1s = smp.tile([128, NT, HEADS], F16, tag="e1s")
            nc.scalar.activation(e1s[:], xls[:], mybir.ActivationFunctionType.Exp)
            e2s = smp.tile([128, NT, HEADS], F16, tag="e2s")
            nc.scalar.activation(e2s[:], xls[:], mybir.ActivationFunctionType.Exp, scale=NEG_SLOPE)
            nc.vector.tensor_tensor(out=wself1[:], in0=e1s[:], in1=e2s[:], op=mybir.AluOpType.max)

            # ---- layer 1
            for gi, (t0, t1) in enumerate(groups):
                SA, SB = gSA[gi], gSB[gi]
                S = SA + SB
                nt = t1 - t0
                c0 = gc0[gi]
                idxg = ixp.tile([128, S * 8], I16, tag="idx")
                nc.scalar.dma_start(out=idxg[:], in_=idx_flat[:, c0 * 8:(c0 + S) * 8])
                G = gp.tile([128, S, T1_COLS], F16, tag="G1")
                if SA:
                    nc.gpsimd.dma_gather(
                        G[:, 0:SA, :], T1_sh[0:b_base, :], idxg[:, 0:SA * 8],
                        128 * SA, 128 * SA, T1_COLS, queue_num=qn(), single_packet=False)
                if SB:
                    nc.gpsimd.dma_gather(
                        G[:, SA:S, :], T1_sh[b_base:nrows, :], idxg[:, SA * 8:S * 8],
                        128 * SB, 128 * SB, T1_COLS, queue_num=qn(), single_packet=False)
                OT = otp.tile([128, S, 128], F16, tag="OT")
                nc.scalar.dma_start(out=OT[:], in_=ohT_d[:, c0:c0 + S, :])
                OH = ohp.tile([128, S, 128], F16, tag="OH")
                nc.vector.tensor_tensor(
                    out=OH[:], in0=codes[:, c0:c0 + S, None].to_broadcast([128, S, 128]),
                    in1=iota[:, None, :].to_broadcast([128, S, 128]),
                    op=mybir.AluOpType.is_equal)

                # group-level alD / weights / messages
                alps = pal.tile([128, 4 * S], F32, tag="al")
                for cs_ in range(S):
                    t = slot_tile[gi][cs_]
                    nc.tensor.matmul(out=alps[:, 4 * cs_:4 * cs_ + 4], lhsT=OT[:, cs_, :],
                                     rhs=alT1[:, t * 8 + 4:t * 8 + 8], start=True, stop=True)
                alDs = smp.tile([128, S, HEADS], F16, tag="alDs")
                nc.vector.tensor_copy(out=alDs[:], in_=alps[:].rearrange("p (k e) -> p k e", e=4))
                xl = smp.tile([128, S, HEADS], F16, tag="xl")
                nc.vector.tensor_tensor(out=xl[:], in0=G[:, :, 128:132], in1=alDs[:],
                                        op=mybir.AluOpType.add)
                e1 = smp.tile([128, S, HEADS], F16, tag="e1")
                nc.scalar.activation(e1[:], xl[:], mybir.ActivationFunctionType.Exp)
                e2 = smp.tile([128, S, HEADS], F16, tag="e2")
                nc.scalar.activation(e2[:], xl[:], mybir.ActivationFunctionType.Exp, scale=NEG_SLOPE)
                rhs = rp.tile([128, S + nt, 132], F16, tag="rhs1")
                nc.vector.tensor_tensor(out=rhs[:, 0:S, 128:132], in0=e1[:], in1=e2[:],
                                        op=mybir.AluOpType.max)
                nc.vector.tensor_tensor(
                    out=rhs[:, 0:S, 0:128].rearrange("p k (h j) -> p k h j", h=4),
                    in0=G[:, :, 0:128].rearrange("p k (h j) -> p k h j", h=4),
                    in1=rhs[:, 0:S, 128:132][:, :, :, None].to_broadcast([128, S, 4, 32]),
                    op=mybir.AluOpType.mult)
                nc.vector.tensor_copy(
                    out=rhs[:, S:S + nt, 128:132], in_=wself1[:, t0:t1, :])
                nc.vector.tensor_tensor(
                    out=rhs[:, S:S + nt, 0:128].rearrange("p k (h j) -> p k h j", h=4),
                    in0=h_own[:, t0 * 128:t1 * 128].rearrange("p (t h j) -> p t h j", t=nt, h=4),
                    in1=wself1[:, t0:t1, :, None].to_broadcast([128, nt, 4, 32]),
                    op=mybir.AluOpType.mult)

                psall = ep.tile([128, nt, 132], F32, tag="psall")
                for ti, t in enumerate(range(t0, t1)):
                    ca, cb = chA[t], chB[t]
                    slots = list(range(aoff[t], aoff[t] + ca)) + \
                            list(range(SA + boff[t], SA + boff[t] + cb))
                    ps = ppa.tile([128, 132], F32, tag="agg")
                    for ci, cs_ in enumerate(slots):
                        nc.tensor.matmul(out=ps[:], lhsT=OH[:, cs_, :], rhs=rhs[:, cs_, :],
                                         start=(ci == 0), stop=False)
                    nc.tensor.matmul(out=ps[:], lhsT=ident[:], rhs=rhs[:, S + ti, :],
                                     start=False, stop=True)
                    nc.vector.tensor_copy(out=psall[:, ti, :], in_=ps[:])
                psg_v = psall[:]

                # group-level epilogue
                rec = smp.tile([128, nt, HEADS], F32, tag="rec")
                nc.vector.reciprocal(out=rec[:], in_=psg_v[:, :, 128:132])
                y1 = ep.tile([128, nt, 128], F16, tag="y")
                nc.vector.tensor_tensor(
                    out=y1[:].rearrange("p t (h j) -> p t h j", h=4),
                    in0=psg_v[:, :, 0:128].rearrange("p t (h j) -> p t h j", h=4),
                    in1=rec[:, :, :, None].to_broadcast([128, nt, 4, 32]),
                    op=mybir.AluOpType.mult)
                nc.vector.tensor_tensor(
                    out=y1[:], in0=y1[:],
                    in1=b1r[:, None, :].to_broadcast([128, nt, 128]),
                    op=mybir.AluOpType.add)
                m1 = ep.tile([128, nt, 128], F16, tag="m1")
                nc.vector.tensor_scalar(out=m1[:], in0=y1[:], scalar1=0.0, scalar2=None,
                                        op0=mybir.AluOpType.min)
                nc.scalar.activation(m1[:], m1[:], mybir.ActivationFunctionType.Exp)
                nc.vector.tensor_scalar(out=y1[:], in0=y1[:], scalar1=0.0, scalar2=-1.0,
                                        op0=mybir.AluOpType.max, op1=mybir.AluOpType.add)
                h2 = m1
                nc.vector.tensor_tensor(out=h2[:], in0=m1[:], in1=y1[:], op=mybir.AluOpType.add)

                t2g = sp.tile([128, nt, T2_COLS], F16, tag="t2b")
                for ti, t in enumerate(range(t0, t1)):
                    pt = ppm.tile([128, 128], F16, tag="misc")
                    nc.tensor.transpose(out=pt[:], in_=h2[:, ti, :], identity=ident[:])
                    h2T = ep.tile([128, 128], F16, tag="h2T")
                    nc.vector.tensor_copy(out=h2T[:], in_=pt[:])
                    psz = ppm.tile([128, HID + 2], F32, tag="misc")
                    nc.tensor.matmul(out=psz[:], lhsT=h2T[:], rhs=W2big[:], start=True, stop=True)
                    nc.vector.tensor_tensor(out=t2g[:, ti, 0:HID + 2], in0=psz[:], in1=b2r[:],
                                            op=mybir.AluOpType.add)
                nc.scalar.dma_start(
                    out=T2_own[t0 * 128:t1 * 128, :].rearrange("(t p) c -> p t c", p=128),
                    in_=t2g[:])
                nc.vector.tensor_copy(
                    out=z_own[:, t0 * HID:t1 * HID].rearrange("p (t c) -> p t c", t=nt),
                    in_=t2g[:, :, 0:HID])
                nc.vector.tensor_copy(
                    out=alT2[:, t0 * 2:t1 * 2].rearrange("p (t c) -> p t c", t=nt),
                    in_=t2g[:, :, HID:HID + 2])
                if t0 < NT // 2 <= t1:
                    allgather(T2_own, T2_sh, 0)
            allgather(T2_own, T2_sh, 1)

            alT2v = alT2[:].rearrange("p (t e) -> p t e", t=NT)
            xls2 = smp.tile([128, NT, 1], F16, tag="xls2")
            nc.vector.tensor_tensor(out=xls2[:], in0=alT2v[:, :, 0:1], in1=alT2v[:, :, 1:2],
                                    op=mybir.AluOpType.add)
            e1s2 = smp.tile([128, NT, 1], F16, tag="e1s2")
            nc.scalar.activation(e1s2[:], xls2[:], mybir.ActivationFunctionType.Exp)
            e2s2 = smp.tile([128, NT, 1], F16, tag="e2s2")
            nc.scalar.activation(e2s2[:], xls2[:], mybir.ActivationFunctionType.Exp, scale=NEG_SLOPE)
            nc.vector.tensor_tensor(out=wself2[:], in0=e1s2[:], in1=e2s2[:], op=mybir.AluOpType.max)

            # ---- layer 2
            for gi, (t0, t1) in enumerate(groups):
                SA, SB = gSA[gi], gSB[gi]
                S = SA + SB
                nt = t1 - t0
                c0 = gc0[gi]
                idxg = ixp.tile([128, S * 8], I16, tag="idx")
                nc.scalar.dma_start(out=idxg[:], in_=idx_flat[:, c0 * 8:(c0 + S) * 8])
                G2 = gp.tile([128, S, T2_COLS], F16, tag="G2")
                if SA:
                    nc.gpsimd.dma_gather(
                        G2[:, 0:SA, :], T2_sh[0:b_base, :], idxg[:, 0:SA * 8],
                        128 * SA, 128 * SA, T2_COLS, queue_num=qn(), single_packet=False)
                if SB:
                    nc.gpsimd.dma_gather(
                        G2[:, SA:S, :], T2_sh[b_base:nrows, :], idxg[:, SA * 8:S * 8],
                        128 * SB, 128 * SB, T2_COLS, queue_num=qn(), single_packet=False)
                OT = otp.tile([128, S, 128], F16, tag="OT")
                nc.scalar.dma_start(out=OT[:], in_=ohT_d[:, c0:c0 + S, :])
                OH = ohp.tile([128, S, 128], F16, tag="OH")
                nc.vector.tensor_tensor(
                    out=OH[:], in0=codes[:, c0:c0 + S, None].to_broadcast([128, S, 128]),
                    in1=iota[:, None, :].to_broadcast([128, S, 128]),
                    op=mybir.AluOpType.is_equal)

                alps2 = pal.tile([128, S], F32, tag="al")
                for cs_ in range(S):
                    t = slot_tile[gi][cs_]
                    nc.tensor.matmul(out=alps2[:, cs_:cs_ + 1], lhsT=OT[:, cs_, :],
                                     rhs=alT2[:, t * 2 + 1:t * 2 + 2], start=True, stop=True)
                alDs2 = smp.tile([128, S, 1], F16, tag="alDs2")
                nc.vector.tensor_copy(out=alDs2[:], in_=alps2[:, :, None])
                xl2 = smp.tile([128, S, 1], F16, tag="xl2")
                nc.vector.tensor_tensor(out=xl2[:], in0=G2[:, :, 32:33], in1=alDs2[:],
                                        op=mybir.AluOpType.add)
                e1b = smp.tile([128, S, 1], F16, tag="e1b")
                nc.scalar.activation(e1b[:], xl2[:], mybir.ActivationFunctionType.Exp)
                e2b = smp.tile([128, S, 1], F16, tag="e2b")
                nc.scalar.activation(e2b[:], xl2[:], mybir.ActivationFunctionType.Exp, scale=NEG_SLOPE)
                rhs2 = rp.tile([128, S + nt, HID + 1], F16, tag="rhs2")
                nc.vector.tensor_tensor(out=rhs2[:, 0:S, HID:HID + 1], in0=e1b[:], in1=e2b[:],
                                        op=mybir.AluOpType.max)
                nc.vector.tensor_tensor(
                    out=rhs2[:, 0:S, 0:HID], in0=G2[:, :, 0:HID],
                    in1=rhs2[:, 0:S, HID:HID + 1].to_broadcast([128, S, HID]),
                    op=mybir.AluOpType.mult)
                nc.vector.tensor_copy(
                    out=rhs2[:, S:S + nt, HID:HID + 1], in_=wself2[:, t0:t1, :])
                nc.vector.tensor_tensor(
                    out=rhs2[:, S:S + nt, 0:HID],
                    in0=z_own[:, t0 * HID:t1 * HID].rearrange("p (t c) -> p t c", t=nt),
                    in1=wself2[:, t0:t1, :].to_broadcast([128, nt, HID]),
                    op=mybir.AluOpType.mult)

                psall2 = ep.tile([128, nt, 33], F32, tag="psall2")
                for ti, t in enumerate(range(t0, t1)):
                    ca, cb = chA[t], chB[t]
                    slots = list(range(aoff[t], aoff[t] + ca)) + \
                            list(range(SA + boff[t], SA + boff[t] + cb))
                    ps2 = ppa.tile([128, 33], F32, tag="agg")
                    for ci, cs_ in enumerate(slots):
                        nc.tensor.matmul(out=ps2[:], lhsT=OH[:, cs_, :], rhs=rhs2[:, cs_, :],
                                         start=(ci == 0), stop=False)
                    nc.tensor.matmul(out=ps2[:], lhsT=ident[:], rhs=rhs2[:, S + ti, :],
                                     start=False, stop=True)
                    nc.vector.tensor_copy(out=psall2[:, ti, :], in_=ps2[:])
                psg2_v = psall2[:]

                rec2 = smp.tile([128, nt, 1], F32, tag="rec2")
                nc.vector.reciprocal(out=rec2[:], in_=psg2_v[:, :, HID:HID + 1])
                y2 = ep.tile([128, nt, HID], F16, tag="y2")
                nc.vector.tensor_tensor(out=y2[:], in0=psg2_v[:, :, 0:HID],
                                        in1=rec2[:].to_broadcast([128, nt, HID]),
                                        op=mybir.AluOpType.mult)
                m2 = ep.tile([128, nt, HID], F16, tag="m2")
                nc.vector.tensor_scalar(out=m2[:], in0=y2[:], scalar1=0.0, scalar2=None,
                                        op0=mybir.AluOpType.min)
                nc.scalar.activation(m2[:], m2[:], mybir.ActivationFunctionType.Exp)
                nc.vector.tensor_scalar(out=y2[:], in0=y2[:], scalar1=0.0, scalar2=-1.0,
                                        op0=mybir.AluOpType.max, op1=mybir.AluOpType.add)
                h3 = m2
                nc.vector.tensor_tensor(out=h3[:], in0=m2[:], in1=y2[:], op=mybir.AluOpType.add)

                outg = ep.tile([128, nt, OUT_CH], F32, tag="outf")
                for ti, t in enumerate(range(t0, t1)):
                    pt2 = ppm.tile([128, 128], F16, tag="misc")
                    nc.tensor.transpose(out=pt2[:HID, :], in_=h3[:, ti, :], identity=ident[:])
                    h3T = ep.tile([HID, 128], F16, tag="h3T")
                    nc.vector.tensor_copy(out=h3T[:], in_=pt2[:HID, :])
                    psf = ppm.tile([128, OUT_CH], F32, tag="misc")
                    nc.tensor.matmul(out=psf[:], lhsT=h3T[:], rhs=Wout[:], start=True, stop=True)
                    nc.vector.tensor_tensor(out=outg[:, ti, :], in0=psf[:], in1=boutr[:],
                                            op=mybir.AluOpType.add)
                nc.scalar.dma_start(
                    out=out_d[t0 * 128:t1 * 128, :].rearrange("(t p) c -> p t c", p=128),
                    in_=outg[:])

    nc.compile()
    return nc


def _run(inputs, trace=False):
    meta, in_maps = _prep(**inputs)
    nc = _build(meta)
    res = run_bass_kernel_spmd(nc, in_maps, core_ids=list(range(N_CORES)), trace=trace)
    outg = np.concatenate([res.results[c]["out"] for c in range(N_CORES)], axis=0)
    out_nodes = np.empty((meta["n_pad"], OUT_CH), np.float32)
    out_nodes[meta["perm_rows"]] = outg
    return out_nodes[:meta["N"]], res


def kernel(**inputs):
    out, _ = _run(inputs, trace=False)
    return out


# revision 17
# speedup vs baseline: 1.2845x; 1.1081x over previous
"""GAT (2-layer, 4-head then 1-head) on 8 Trainium2 NeuronCores.

Strategy (v3 — dense one-hot chunks, group-level batching)
----------------------------------------------------------
- Nodes degree-sorted and dealt round-robin to 8 cores; each core's 5120
  nodes form 40 dst tiles of 128.
- Edges (self-loops excluded) are packed DENSELY per (tile, table-half)
  into 128-edge chunks (pad ~9%). Aggregation multiplies each chunk by a
  one-hot lhsT whose column d selects the partitions holding edges of
  dst d, accumulating Sum_e w_e*[h_e | 1] per dst in PSUM.
- Per-edge al_dst: alD[p] = onehotT_c (contract dst) al8_own — computed
  for ALL chunks of a group into one PSUM bank, one matmul per chunk.
  onehotT is streamed from DRAM; the aggregation one-hot is generated on
  device (DVE is_equal(codes, iota)).
- All per-edge DVE work (al add, exp, max, message multiply) happens at
  GROUP granularity (~36 chunks per instruction), not per tile; only the
  aggregation matmuls and the epilogue are per tile.
- Self-loops never gathered: extra rhs slots + identity-matmul chunks
  fed from SBUF-resident h_own / z_own.
- Softmax without max-subtraction (exp(lrelu(x)) = max(exp x, exp .2x));
  denominators ride the same one-hot matmul (w columns). Pad slots have
  all-zero one-hot columns.
- Node tables: L1 rows [h(128)|al_src(4)|pad] 512B; L2 rows
  [z+b2(32)|as2(1)|pad] 256B. Table rows are numbered CHUNK-MAJOR
  (pos<2560 first for all cores, then the rest) so each half-table
  AllGather has a contiguous output and can overlap compute. int16
  gather indices via the A/B table split at row 30721.
"""

import numpy as np

import concourse.bacc as bacc
import concourse.mybir as mybir
import concourse.tile as tile
from concourse.bass_utils import run_bass_kernel_spmd

F32 = mybir.dt.float32
F16 = mybir.dt.float16
I16 = mybir.dt.int16

IN_CH = 128
HID = 32
HEADS = 4
OUT_CH = 112
NEG_SLOPE = 0.2

T1_COLS = 256
T2_COLS = 128

N_CORES = 8
GCAP = 36
PAD_CODE = 200
HROWS = 2560  # rows per core per allgather chunk (2 chunks)


def _prep(x, edge_index, W1, a_src1, a_dst1, b1, W2, a_src2, a_dst2, b2, W_out, b_out):
    N = x.shape[0]
    per_core = -(-N // (N_CORES * 128)) * 128
    n_pad = per_core * N_CORES
    NT = per_core // 128
    nrows = n_pad + 2
    b_base = 1 + N_CORES * HROWS
    assert b_base - 1 <= 32767 and nrows - b_base <= 32767
    assert per_core == 2 * HROWS

    src = np.asarray(edge_index[0], np.int64)
    dst = np.asarray(edge_index[1], np.int64)
    E = src.shape[0]

    deg = np.bincount(dst, minlength=n_pad)
    order = np.argsort(deg, kind="stable")
    rank = np.empty(n_pad, np.int64)
    rank[order] = np.arange(n_pad)
    coreid = rank % N_CORES
    pos = rank // N_CORES
    grow = coreid * per_core + pos
    # chunk-major table rows: all cores' pos<HROWS first, then the rest
    trow = 1 + (pos // HROWS) * (N_CORES * HROWS) + coreid * HROWS + pos % HROWS
    perm_rows = np.empty(n_pad, np.int64)
    perm_rows[grow] = np.arange(n_pad)

    sr = trow[src]
    dr = grow[dst]
    gB = sr >= b_base
    core = dr // per_core
    tl = (dr % per_core) // 128
    lane = dr % 128

    EaT = np.zeros((N_CORES, NT), np.int64)
    EbT = np.zeros((N_CORES, NT), np.int64)
    np.add.at(EaT, (core[~gB], tl[~gB]), 1)
    np.add.at(EbT, (core[gB], tl[gB]), 1)
    chA = (-(-EaT // 128)).max(axis=0)
    chB = (-(-EbT // 128)).max(axis=0)

    groups = []
    t = 0
    while t < NT:
        e = t
        tot = 0
        while e < NT and (e == t or tot + chA[e] + chB[e] <= GCAP):
            tot += chA[e] + chB[e]
            e += 1
        groups.append((t, e))
        t = e

    aoff = np.zeros(NT, np.int64)
    boff = np.zeros(NT, np.int64)
    gc0 = []
    gSA = []
    gSB = []
    C = 0
    g_of_tile = np.zeros(NT, np.int64)
    for gi, (t0, t1) in enumerate(groups):
        sa = int(chA[t0:t1].sum())
        sb = int(chB[t0:t1].sum())
        gc0.append(C)
        gSA.append(sa)
        gSB.append(sb)
        off = 0
        for t in range(t0, t1):
            g_of_tile[t] = gi
            aoff[t] = off
            off += chA[t]
        off = 0
        for t in range(t0, t1):
            boff[t] = off
            off += chB[t]
        C += sa + sb
    totidx = C * 128
    assert totidx % 16 == 0

    SENT_A = 0
    SENT_B = nrows - 1 - b_base
    idx_streams = np.zeros((N_CORES, C, 128), np.int16)
    for gi in range(len(groups)):
        idx_streams[:, gc0[gi]:gc0[gi] + gSA[gi], :] = SENT_A
        idx_streams[:, gc0[gi] + gSA[gi]:gc0[gi] + gSA[gi] + gSB[gi], :] = SENT_B
    codes_streams = np.full((N_CORES, C, 128), PAD_CODE, np.int16)

    cbaseA = np.array([gc0[g_of_tile[t]] + aoff[t] for t in range(NT)])
    cbaseB = np.array([gc0[g_of_tile[t]] + gSA[g_of_tile[t]] + boff[t] for t in range(NT)])

    key = (core * NT + tl) * 2 + gB.astype(np.int64)
    eorder = np.argsort(key, kind="stable")
    ks = key[eorder]
    newrun = np.ones(E, bool)
    newrun[1:] = ks[1:] != ks[:-1]
    run_start = np.flatnonzero(newrun)
    run_id = np.cumsum(newrun) - 1
    j = np.arange(E) - run_start[run_id]
    cs = core[eorder]
    tls = tl[eorder]
    gs = gB[eorder]
    cidx = np.where(gs, cbaseB[tls], cbaseA[tls]) + j // 128
    idx_streams[cs, cidx, j % 128] = np.where(gs, sr[eorder] - b_base, sr[eorder]).astype(np.int16)
    codes_streams[cs, cidx, j % 128] = lane[eorder]

    idx_wrapped = np.empty((N_CORES, 128, totidx // 16), np.int16)
    for c in range(N_CORES):
        w16 = idx_streams[c].reshape(-1, 16).T
        idx_wrapped[c] = np.tile(w16, (8, 1))

    f16 = np.float16
    codes_pc = np.transpose(codes_streams, (0, 2, 1)).astype(f16)
    d_ar = np.arange(128, dtype=np.int16)[:, None, None]
    ohT = np.empty((N_CORES, 128, C, 128), f16)
    for c in range(N_CORES):
        ohT[c] = (codes_streams[c][None, :, :] == d_ar).astype(f16)

    xp = np.zeros((n_pad, IN_CH), np.float32)
    xp[:N] = np.asarray(x, np.float32)
    x_slices = np.empty((N_CORES, IN_CH, per_core), np.float32)
    for c in range(N_CORES):
        x_slices[c] = xp[perm_rows[c * per_core:(c + 1) * per_core]].T

    W1 = np.asarray(W1, np.float32)
    Bsrc = np.zeros((HEADS * HID, HEADS), np.float32)
    Bdst = np.zeros((HEADS * HID, HEADS), np.float32)
    for h in range(HEADS):
        Bsrc[h * HID:(h + 1) * HID, h] = np.asarray(a_src1[h], np.float32)
        Bdst[h * HID:(h + 1) * HID, h] = np.asarray(a_dst1[h], np.float32)
    W1big = np.concatenate([W1, W1 @ Bsrc, W1 @ Bdst], axis=1)
    W2 = np.asarray(W2, np.float32)
    W2big = np.concatenate(
        [W2, W2 @ np.asarray(a_src2, np.float32).T, W2 @ np.asarray(a_dst2, np.float32).T],
        axis=1,
    )
    b1_rep = np.tile(np.asarray(b1, np.float32)[None, :], (128, 1))
    b2_rep = np.zeros((128, HID + 2), np.float32)
    b2_rep[:, :HID] = np.asarray(b2, np.float32)[None, :]
    bout_rep = np.tile(np.asarray(b_out, np.float32)[None, :], (128, 1))
    ident = np.eye(128, dtype=f16)
    iota = np.tile(np.arange(128, dtype=f16)[None, :], (128, 1))

    meta = dict(
        N=N, n_pad=n_pad, per_core=per_core, NT=NT, nrows=nrows, b_base=b_base,
        chA=chA.tolist(), chB=chB.tolist(), groups=groups, gc0=gc0, gSA=gSA,
        gSB=gSB, aoff=aoff.tolist(), boff=boff.tolist(), C=C, totidx=totidx,
        perm_rows=perm_rows,
    )
    shared = dict(
        W1big=W1big, W2big=W2big.astype(f16), Wout=np.asarray(W_out, np.float32).astype(f16),
        b1_rep=b1_rep, b2_rep=b2_rep, bout_rep=bout_rep, ident=ident, iota=iota,
    )
    in_maps = []
    for c in range(N_CORES):
        m = dict(shared)
        m["x_slice"] = np.ascontiguousarray(x_slices[c])
        m["idx_flat"] = np.ascontiguousarray(idx_wrapped[c])
        m["codes"] = np.ascontiguousarray(codes_pc[c])
        m["ohT"] = np.ascontiguousarray(ohT[c])
        in_maps.append(m)
    return meta, in_maps


def _build(meta):
    per_core, NT, nrows, b_base = meta["per_core"], meta["NT"], meta["nrows"], meta["b_base"]
    chA, chB = meta["chA"], meta["chB"]
    groups, gc0, gSA, gSB = meta["groups"], meta["gc0"], meta["gSA"], meta["gSB"]
    aoff, boff, C = meta["aoff"], meta["boff"], meta["C"]
    totidx = meta["totidx"]

    nc = bacc.Bacc("TRN2", num_devices=N_CORES, num_swdge_queues=4,
                   dynamic_dma_scratch_size=32768)

    x_slice = nc.dram_tensor("x_slice", [IN_CH, per_core], F32, kind="ExternalInput")
    idx_flat = nc.dram_tensor("idx_flat", [128, totidx // 16], I16, kind="ExternalInput")
    codes_d = nc.dram_tensor("codes", [128, C], F16, kind="ExternalInput")
    ohT_d = nc.dram_tensor("ohT", [128, C, 128], F16, kind="ExternalInput")
    W1big_d = nc.dram_tensor("W1big", [128, 136], F32, kind="ExternalInput")
    W2big_d = nc.dram_tensor("W2big", [128, HID + 2], F16, kind="ExternalInput")
    Wout_d = nc.dram_tensor("Wout", [HID, OUT_CH], F16, kind="ExternalInput")
    b1_d = nc.dram_tensor("b1_rep", [128, 128], F32, kind="ExternalInput")
    b2_d = nc.dram_tensor("b2_rep", [128, HID + 2], F32, kind="ExternalInput")
    bout_d = nc.dram_tensor("bout_rep", [128, OUT_CH], F32, kind="ExternalInput")
    ident_d = nc.dram_tensor("ident", [128, 128], F16, kind="ExternalInput")
    iota_d = nc.dram_tensor("iota", [128, 128], F16, kind="ExternalInput")

    T1_own = nc.dram_tensor("T1_own", [per_core, T1_COLS], F16, kind="Internal")
    T1_sh = nc.dram_tensor("T1_sh", [nrows, T1_COLS], F16, kind="Internal", addr_space="Shared")
    T2_own = nc.dram_tensor("T2_own", [per_core, T2_COLS], F16, kind="Internal")
    T2_sh = nc.dram_tensor("T2_sh", [nrows, T2_COLS], F16, kind="Internal", addr_space="Shared")
    out_d = nc.dram_tensor("out", [per_core, OUT_CH], F32, kind="ExternalOutput")

    rgroups = [list(range(N_CORES))]
    qctr = [0]

    def qn():
        q = qctr[0] % 4
        qctr[0] += 1
        return q

    def allgather(own, sh, half):
        r0 = half * HROWS
        o0 = 1 + half * N_CORES * HROWS
        nc.gpsimd.collective_compute(
            "AllGather", mybir.AluOpType.bypass, replica_groups=rgroups,
            ins=[own[r0:r0 + HROWS, :]], outs=[sh[o0:o0 + N_CORES * HROWS, :]],
        )

    # per-group slot -> tile map
    slot_tile = []
    for gi, (t0, t1) in enumerate(groups):
        st = []
        for t in range(t0, t1):
            st += [t] * chA[t]
        for t in range(t0, t1):
            st += [t] * chB[t]
        slot_tile.append(st)

    with tile.TileContext(nc) as tc:
        with (
            tc.tile_pool(name="const", bufs=1) as cp,
            tc.tile_pool(name="persist", bufs=1) as pp,
            tc.tile_pool(name="xa", bufs=2) as xap,
            tc.tile_pool(name="stage", bufs=3) as sp,
            tc.tile_pool(name="idxp", bufs=3) as ixp,
            tc.tile_pool(name="gath", bufs=3) as gp,
            tc.tile_pool(name="ot", bufs=2) as otp,
            tc.tile_pool(name="oh", bufs=2) as ohp,
            tc.tile_pool(name="rhs", bufs=2) as rp,
            tc.tile_pool(name="small", bufs=3) as smp,
            tc.tile_pool(name="epi", bufs=2) as ep,
            tc.tile_pool(name="psa", bufs=3, space="PSUM") as ppa,
            tc.tile_pool(name="psal", bufs=2, space="PSUM") as pal,
            tc.tile_pool(name="psm", bufs=3, space="PSUM") as ppm,
        ):
            # ---- consts
            W1big = cp.tile([128, 136], F32)
            nc.sync.dma_start(out=W1big[:], in_=W1big_d[:])
            W2big = cp.tile([128, HID + 2], F16)
            nc.sync.dma_start(out=W2big[:], in_=W2big_d[:])
            Wout = cp.tile([HID, OUT_CH], F16)
            nc.sync.dma_start(out=Wout[:], in_=Wout_d[:])
            b1r = cp.tile([128, 128], F32)
            nc.sync.dma_start(out=b1r[:], in_=b1_d[:])
            b2r = cp.tile([128, HID + 2], F32)
            nc.sync.dma_start(out=b2r[:], in_=b2_d[:])
            boutr = cp.tile([128, OUT_CH], F32)
            nc.sync.dma_start(out=boutr[:], in_=bout_d[:])
            ident = cp.tile([128, 128], F16)
            nc.sync.dma_start(out=ident[:], in_=ident_d[:])
            iota = cp.tile([128, 128], F16)
            nc.sync.dma_start(out=iota[:], in_=iota_d[:])
            codes = cp.tile([128, C], F16)
            nc.sync.dma_start(out=codes[:], in_=codes_d[:])

            h_own = pp.tile([128, NT * 128], F16)
            alT1 = pp.tile([128, NT * 8], F16)
            z_own = pp.tile([128, NT * HID], F16)
            alT2 = pp.tile([128, NT * 2], F16)
            wself1 = pp.tile([128, NT, HEADS], F16)
            wself2 = pp.tile([128, NT, 1], F16)

            zs1 = cp.tile([1, T1_COLS], F16)
            nc.vector.memset(zs1[:], 0.0)
            nc.sync.dma_start(out=T1_sh[0:1, :], in_=zs1[:])
            nc.sync.dma_start(out=T1_sh[nrows - 1:nrows, :], in_=zs1[:])
            nc.sync.dma_start(out=T2_sh[0:1, :], in_=zs1[:, 0:T2_COLS])
            nc.sync.dma_start(out=T2_sh[nrows - 1:nrows, :], in_=zs1[:, 0:T2_COLS])

            # ---- phase A
            for t in range(NT):
                xa = xap.tile([128, 128], F32)
                nc.sync.dma_start(out=xa[:], in_=x_slice[:, t * 128:(t + 1) * 128])
                ps = ppa.tile([128, 136], F32, tag="agg")
                nc.tensor.matmul(out=ps[:], lhsT=xa[:], rhs=W1big[:], start=True, stop=True)
                hb = sp.tile([128, T1_COLS], F16, tag="hb")
                nc.vector.tensor_copy(out=hb[:, 0:132], in_=ps[:, 0:132])
                nc.scalar.dma_start(out=T1_own[t * 128:(t + 1) * 128, :], in_=hb[:])
                nc.vector.tensor_copy(out=h_own[:, t * 128:(t + 1) * 128], in_=ps[:, 0:128])
                nc.vector.tensor_copy(out=alT1[:, t * 8:t * 8 + 8], in_=ps[:, 128:136])
                if t == NT // 2 - 1:
                    allgather(T1_own, T1_sh, 0)
            allgather(T1_own, T1_sh, 1)

            alT1v = alT1[:].rearrange("p (t e) -> p t e", t=NT)
            xls = smp.tile([128, NT, HEADS], F16, tag="xls")
            nc.vector.tensor_tensor(out=xls[:], in0=alT1v[:, :, 0:4], in1=alT1v[:, :, 4:8],
                                    op=mybir.AluOpType.add)
            e1s = smp.tile([128, NT, HEADS], F16, tag="e1s")
            nc.scalar.activation(e1s[:], xls[:], mybir.ActivationFunctionType.Exp)
            e2s = smp.tile([128, NT, HEADS], F16, tag="e2s")
            nc.scalar.activation(e2s[:], xls[:], mybir.ActivationFunctionType.Exp, scale=NEG_SLOPE)
            nc.vector.tensor_tensor(out=wself1[:], in0=e1s[:], in1=e2s[:], op=mybir.AluOpType.max)

            # ---- layer 1
            for gi, (t0, t1) in enumerate(groups):
                SA, SB = gSA[gi], gSB[gi]
                S = SA + SB
                nt = t1 - t0
                c0 = gc0[gi]
                idxg = ixp.tile([128, S * 8], I16, tag="idx")
                nc.sync.dma_start(out=idxg[:], in_=idx_flat[:, c0 * 8:(c0 + S) * 8])
                G = gp.tile([128, S, T1_COLS], F16, tag="G1")
                if SA:
                    nc.gpsimd.dma_gather(
                        G[:, 0:SA, :], T1_sh[0:b_base, :], idxg[:, 0:SA * 8],
                        128 * SA, 128 * SA, T1_COLS, queue_num=qn(), single_packet=False)
                if SB:
                    nc.gpsimd.dma_gather(
                        G[:, SA:S, :], T1_sh[b_base:nrows, :], idxg[:, SA * 8:S * 8],
                        128 * SB, 128 * SB, T1_COLS, queue_num=qn(), single_packet=False)
                OT = otp.tile([128, S, 128], F16, tag="OT")
                nc.scalar.dma_start(out=OT[:], in_=ohT_d[:, c0:c0 + S, :])
                OH = ohp.tile([128, S, 128], F16, tag="OH")
                nc.vector.tensor_tensor(
                    out=OH[:], in0=codes[:, c0:c0 + S, None].to_broadcast([128, S, 128]),
                    in1=iota[:, None, :].to_broadcast([128, S, 128]),
                    op=mybir.AluOpType.is_equal)

                # group-level alD / weights / messages
                alps = pal.tile([128, 4 * S], F32, tag="al")
                for cs_ in range(S):
                    t = slot_tile[gi][cs_]
                    nc.tensor.matmul(out=alps[:, 4 * cs_:4 * cs_ + 4], lhsT=OT[:, cs_, :],
                                     rhs=alT1[:, t * 8 + 4:t * 8 + 8], start=True, stop=True)
                alDs = smp.tile([128, S, HEADS], F16, tag="alDs")
                nc.vector.tensor_copy(out=alDs[:], in_=alps[:].rearrange("p (k e) -> p k e", e=4))
                xl = smp.tile([128, S, HEADS], F16, tag="xl")
                nc.vector.tensor_tensor(out=xl[:], in0=G[:, :, 128:132], in1=alDs[:],
                                        op=mybir.AluOpType.add)
                e1 = smp.tile([128, S, HEADS], F16, tag="e1")
                nc.scalar.activation(e1[:], xl[:], mybir.ActivationFunctionType.Exp)
                e2 = smp.tile([128, S, HEADS], F16, tag="e2")
                nc.scalar.activation(e2[:], xl[:], mybir.ActivationFunctionType.Exp, scale=NEG_SLOPE)
                rhs = rp.tile([128, S + nt, 132], F16, tag="rhs1")
                nc.vector.tensor_tensor(out=rhs[:, 0:S, 128:132], in0=e1[:], in1=e2[:],
                                        op=mybir.AluOpType.max)
                nc.vector.tensor_tensor(
                    out=rhs[:, 0:S, 0:128].rearrange("p k (h j) -> p k h j", h=4),
                    in0=G[:, :, 0:128].rearrange("p k (h j) -> p k h j", h=4),
                    in1=rhs[:, 0:S, 128:132][:, :, :, None].to_broadcast([128, S, 4, 32]),
                    op=mybir.AluOpType.mult)
                nc.vector.tensor_copy(
                    out=rhs[:, S:S + nt, 128:132], in_=wself1[:, t0:t1, :])
                nc.vector.tensor_tensor(
                    out=rhs[:, S:S + nt, 0:128].rearrange("p k (h j) -> p k h j", h=4),
                    in0=h_own[:, t0 * 128:t1 * 128].rearrange("p (t h j) -> p t h j", t=nt, h=4),
                    in1=wself1[:, t0:t1, :, None].to_broadcast([128, nt, 4, 32]),
                    op=mybir.AluOpType.mult)

                psall = ep.tile([128, nt, 132], F32, tag="psall")
                for ti, t in enumerate(range(t0, t1)):
                    ca, cb = chA[t], chB[t]
                    slots = list(range(aoff[t], aoff[t] + ca)) + \
                            list(range(SA + boff[t], SA + boff[t] + cb))
                    ps = ppa.tile([128, 132], F32, tag="agg")
                    for ci, cs_ in enumerate(slots):
                        nc.tensor.matmul(out=ps[:], lhsT=OH[:, cs_, :], rhs=rhs[:, cs_, :],
                                         start=(ci == 0), stop=False)
                    nc.tensor.matmul(out=ps[:], lhsT=ident[:], rhs=rhs[:, S + ti, :],
                                     start=False, stop=True)
                    nc.vector.tensor_copy(out=psall[:, ti, :], in_=ps[:])
                psg_v = psall[:]

                # group-level epilogue
                rec = smp.tile([128, nt, HEADS], F32, tag="rec")
                nc.vector.reciprocal(out=rec[:], in_=psg_v[:, :, 128:132])
                y1 = ep.tile([128, nt, 128], F16, tag="y")
                nc.vector.tensor_tensor(
                    out=y1[:].rearrange("p t (h j) -> p t h j", h=4),
                    in0=psg_v[:, :, 0:128].rearrange("p t (h j) -> p t h j", h=4),
                    in1=rec[:, :, :, None].to_broadcast([128, nt, 4, 32]),
                    op=mybir.AluOpType.mult)
                nc.vector.tensor_tensor(
                    out=y1[:], in0=y1[:],
                    in1=b1r[:, None, :].to_broadcast([128, nt, 128]),
                    op=mybir.AluOpType.add)
                m1 = ep.tile([128, nt, 128], F16, tag="m1")
                nc.vector.tensor_scalar(out=m1[:], in0=y1[:], scalar1=0.0, scalar2=None,
                                        op0=mybir.AluOpType.min)
                nc.scalar.activation(m1[:], m1[:], mybir.ActivationFunctionType.Exp)
                nc.vector.tensor_scalar(out=y1[:], in0=y1[:], scalar1=0.0, scalar2=-1.0,
                                        op0=mybir.AluOpType.max, op1=mybir.AluOpType.add)
                h2 = m1
                nc.vector.tensor_tensor(out=h2[:], in0=m1[:], in1=y1[:], op=mybir.AluOpType.add)

                t2g = sp.tile([128, nt, T2_COLS], F16, tag="t2b")
                for ti, t in enumerate(range(t0, t1)):
                    pt = ppm.tile([128, 128], F16, tag="misc")
                    nc.tensor.transpose(out=pt[:], in_=h2[:, ti, :], identity=ident[:])
                    h2T = ep.tile([128, 128], F16, tag="h2T")
                    nc.vector.tensor_copy(out=h2T[:], in_=pt[:])
                    psz = ppm.tile([128, HID + 2], F32, tag="misc")
                    nc.tensor.matmul(out=psz[:], lhsT=h2T[:], rhs=W2big[:], start=True, stop=True)
                    nc.vector.tensor_tensor(out=t2g[:, ti, 0:HID + 2], in0=psz[:], in1=b2r[:],
                                            op=mybir.AluOpType.add)
                nc.scalar.dma_start(
                    out=T2_own[t0 * 128:t1 * 128, :].rearrange("(t p) c -> p t c", p=128),
                    in_=t2g[:])
                nc.vector.tensor_copy(
                    out=z_own[:, t0 * HID:t1 * HID].rearrange("p (t c) -> p t c", t=nt),
                    in_=t2g[:, :, 0:HID])
                nc.vector.tensor_copy(
                    out=alT2[:, t0 * 2:t1 * 2].rearrange("p (t c) -> p t c", t=nt),
                    in_=t2g[:, :, HID:HID + 2])
                if t0 < NT // 2 <= t1:
                    allgather(T2_own, T2_sh, 0)
            allgather(T2_own, T2_sh, 1)

            alT2v = alT2[:].rearrange("p (t e) -> p t e", t=NT)
            xls2 = smp.tile([128, NT, 1], F16, tag="xls2")
            nc.vector.tensor_tensor(out=xls2[:], in0=alT2v[:, :, 0:1], in1=alT2v[:, :, 1:2],
                                    op=mybir.AluOpType.add)
            e1s2 = smp.tile([128, NT, 1], F16, tag="e1s2")
            nc.scalar.activation(e1s2[:], xls2[:], mybir.ActivationFunctionType.Exp)
            e2s2 = smp.tile([128, NT, 1], F16, tag="e2s2")
            nc.scalar.activation(e2s2[:], xls2[:], mybir.ActivationFunctionType.Exp, scale=NEG_SLOPE)
            nc.vector.tensor_tensor(out=wself2[:], in0=e1s2[:], in1=e2s2[:], op=mybir.AluOpType.max)

            # ---- layer 2
            for gi, (t0, t1) in enumerate(groups):
                SA, SB = gSA[gi], gSB[gi]
                S = SA + SB
                nt = t1 - t0
                c0 = gc0[gi]
                idxg = ixp.tile([128, S * 8], I16, tag="idx")
                nc.sync.dma_start(out=idxg[:], in_=idx_flat[:, c0 * 8:(c0 + S) * 8])
                G2 = gp.tile([128, S, T2_COLS], F16, tag="G2")
                if SA:
                    nc.gpsimd.dma_gather(
                        G2[:, 0:SA, :], T2_sh[0:b_base, :], idxg[:, 0:SA * 8],
                        128 * SA, 128 * SA, T2_COLS, queue_num=qn(), single_packet=False)
                if SB:
                    nc.gpsimd.dma_gather(
                        G2[:, SA:S, :], T2_sh[b_base:nrows, :], idxg[:, SA * 8:S * 8],
                        128 * SB, 128 * SB, T2_COLS, queue_num=qn(), single_packet=False)
                OT = otp.tile([128, S, 128], F16, tag="OT")
                nc.scalar.dma_start(out=OT[:], in_=ohT_d[:, c0:c0 + S, :])
                OH = ohp.tile([128, S, 128], F16, tag="OH")
                nc.vector.tensor_tensor(
                    out=OH[:], in0=codes[:, c0:c0 + S, None].to_broadcast([128, S, 128]),
                    in1=iota[:, None, :].to_broadcast([128, S, 128]),
                    op=mybir.AluOpType.is_equal)

                alps2 = pal.tile([128, S], F32, tag="al")
                for cs_ in range(S):
                    t = slot_tile[gi][cs_]
                    nc.tensor.matmul(out=alps2[:, cs_:cs_ + 1], lhsT=OT[:, cs_, :],
                                     rhs=alT2[:, t * 2 + 1:t * 2 + 2], start=True, stop=True)
                alDs2 = smp.tile([128, S, 1], F16, tag="alDs2")
                nc.vector.tensor_copy(out=alDs2[:], in_=alps2[:, :, None])
                xl2 = smp.tile([128, S, 1], F16, tag="xl2")
                nc.vector.tensor_tensor(out=xl2[:], in0=G2[:, :, 32:33], in1=alDs2[:],
                                        op=mybir.AluOpType.add)
                e1b = smp.tile([128, S, 1], F16, tag="e1b")
                nc.scalar.activation(e1b[:], xl2[:], mybir.ActivationFunctionType.Exp)
                e2b = smp.tile([128, S, 1], F16, tag="e2b")
                nc.scalar.activation(e2b[:], xl2[:], mybir.ActivationFunctionType.Exp, scale=NEG_SLOPE)
                rhs2 = rp.tile([128, S + nt, HID + 1], F16, tag="rhs2")
                nc.vector.tensor_tensor(out=rhs2[:, 0:S, HID:HID + 1], in0=e1b[:], in1=e2b[:],
                                        op=mybir.AluOpType.max)
                nc.vector.tensor_tensor(
                    out=rhs2[:, 0:S, 0:HID], in0=G2[:, :, 0:HID],
                    in1=rhs2[:, 0:S, HID:HID + 1].to_broadcast([128, S, HID]),
                    op=mybir.AluOpType.mult)
                nc.vector.tensor_copy(
                    out=rhs2[:, S:S + nt, HID:HID + 1], in_=wself2[:, t0:t1, :])
                nc.vector.tensor_tensor(
                    out=rhs2[:, S:S + nt, 0:HID],
                    in0=z_own[:, t0 * HID:t1 * HID].rearrange("p (t c) -> p t c", t=nt),
                    in1=wself2[:, t0:t1, :].to_broadcast([128, nt, HID]),
                    op=mybir.AluOpType.mult)

                psall2 = ep.tile([128, nt, 33], F32, tag="psall2")
                for ti, t in enumerate(range(t0, t1)):
                    ca, cb = chA[t], chB[t]
                    slots = list(range(aoff[t], aoff[t] + ca)) + \
                            list(range(SA + boff[t], SA + boff[t] + cb))
                    ps2 = ppa.tile([128, 33], F32, tag="agg")
                    for ci, cs_ in enumerate(slots):
                        nc.tensor.matmul(out=ps2[:], lhsT=OH[:, cs_, :], rhs=rhs2[:, cs_, :],
                                         start=(ci == 0), stop=False)
                    nc.tensor.matmul(out=ps2[:], lhsT=ident[:], rhs=rhs2[:, S + ti, :],
                                     start=False, stop=True)
                    nc.vector.tensor_copy(out=psall2[:, ti, :], in_=ps2[:])
                psg2_v = psall2[:]

                rec2 = smp.tile([128, nt, 1], F32, tag="rec2")
                nc.vector.reciprocal(out=rec2[:], in_=psg2_v[:, :, HID:HID + 1])
                y2 = ep.tile([128, nt, HID], F16, tag="y2")
                nc.vector.tensor_tensor(out=y2[:], in0=psg2_v[:, :, 0:HID],
                                        in1=rec2[:].to_broadcast([128, nt, HID]),
                                        op=mybir.AluOpType.mult)
                m2 = ep.tile([128, nt, HID], F16, tag="m2")
                nc.vector.tensor_scalar(out=m2[:], in0=y2[:], scalar1=0.0, scalar2=None,
                                        op0=mybir.AluOpType.min)
                nc.scalar.activation(m2[:], m2[:], mybir.ActivationFunctionType.Exp)
                nc.vector.tensor_scalar(out=y2[:], in0=y2[:], scalar1=0.0, scalar2=-1.0,
                                        op0=mybir.AluOpType.max, op1=mybir.AluOpType.add)
                h3 = m2
                nc.vector.tensor_tensor(out=h3[:], in0=m2[:], in1=y2[:], op=mybir.AluOpType.add)

                outg = ep.tile([128, nt, OUT_CH], F32, tag="outf")
                for ti, t in enumerate(range(t0, t1)):
                    pt2 = ppm.tile([128, 128], F16, tag="misc")
                    nc.tensor.transpose(out=pt2[:HID, :], in_=h3[:, ti, :], identity=ident[:])
                    h3T = ep.tile([HID, 128], F16, tag="h3T")
                    nc.vector.tensor_copy(out=h3T[:], in_=pt2[:HID, :])
                    psf = ppm.tile([128, OUT_CH], F32, tag="misc")
                    nc.tensor.matmul(out=psf[:], lhsT=h3T[:], rhs=Wout[:], start=True, stop=True)
                    nc.vector.tensor_tensor(out=outg[:, ti, :], in0=psf[:], in1=boutr[:],
                                            op=mybir.AluOpType.add)
                nc.scalar.dma_start(
                    out=out_d[t0 * 128:t1 * 128, :].rearrange("(t p) c -> p t c", p=128),
                    in_=outg[:])

    nc.compile()
    return nc


def _run(inputs, trace=False):
    meta, in_maps = _prep(**inputs)
    nc = _build(meta)
    res = run_bass_kernel_spmd(nc, in_maps, core_ids=list(range(N_CORES)), trace=trace)
    outg = np.concatenate([res.results[c]["out"] for c in range(N_CORES)], axis=0)
    out_nodes = np.empty((meta["n_pad"], OUT_CH), np.float32)
    out_nodes[meta["perm_rows"]] = outg
    return out_nodes[:meta["N"]], res


def kernel(**inputs):
    out, _ = _run(inputs, trace=False)
    return out
